# revision 29
# baseline (speedup 1.0000x reference)
"""Trainium2 Bass kernel for nn_Attention_84602265797043 (gnn_message_passing).

Math (per (b, m) slice of x [B=8, M=12, N=32, D=256]):
  adj  = pearson_corr(x_bm)                    # [N, N]; only its SIGN is used
  e0   = (s_i + 2*s_j)/(2D) - diag(s_i/(2D))   # s = row sums; rank-2 + diag
  ef   = e0 flattened to the C=N*N channel dim # [C, M] per b
  h    = relu(W1 @ ef);  e2 = W2 @ h           # squeeze-excite MLP over C
  e    = sigmoid(e2) reshaped [N, N]
  att  = softmax(where(adj > 0, e, -1e12), axis=-1)
  out  = att @ x_bm

Device-side simplifications (all validated against the reference on CPU):
  * adj > 0  <=>  cov > 0, cov = (x-mean)@(x-mean).T = x@x.T - s s.T/D. The
    Gram part comes from PE-transposed x; the rank-1 mean term is a third,
    K=1 accumulating matmul, so no mean-subtracted copy of x is needed.
  * e0 uses the rank-2 + diag form (exact up to 6 of ~25M float-equality
    collisions; ~1e-6 relative on the output).
  * ef = Phat @ s_m with constant Phat [C, N], so h = relu(U @ s_m) with
    U = W1 @ Phat computed on device. The MLP runs in bf16 (~8e-6 relative
    on the output; sigmoid+masking crush the quantization error).
  * sigmoid outputs lie in (0,1), so softmax needs no max-subtraction:
    p = exp(e) * (cov > 0). p is built directly in transposed (lhsT) layout
    (cov is symmetric, so the same mask tile serves), block-diagonal over 4
    slices, and the row-sum rides the final matmul as an appended ones
    column of x: out = (p.T-matmul [x | 1])[:, :D] scaled by 1/rowsum.

Sharding: data-parallel over B - core b gets x[b], bf16 W1T/W2T (replicated,
host-pre-transposed so every DMA is contiguous), and produces out[b].
Weight DMAs are split across both HWDGE queues (sync + scalar engines).

Implementation: raw Bass (this walrus build accepts only one attached
sync-wait per instruction; TileContext-emitted BIR is rejected), standalone
wait_ge instructions, and per-engine programs emitted from one ordered list
with precomputed semaphore checkpoints. Compute-engine sem updates ride on
DRAIN instructions (engine pipelines retire sem updates before writes land).
"""

from contextlib import ExitStack

import ml_dtypes
import numpy as np

import concourse.bass as bass
import concourse.mybir as mybir
from concourse.bass_utils import run_bass_kernel_spmd

F32 = mybir.dt.float32
BF16 = mybir.dt.bfloat16
B, M, N, D = 8, 12, 32, 256
C = N * N            # 1024 = MLP channel dim
HID = C // 2         # 512  = MLP hidden dim
MG = 4               # m-slices per partition group (MG*N = 128)
NGRP = M // MG       # 3 groups
X = mybir.AxisListType.X
AF = mybir.ActivationFunctionType
DEBUG = False


def _phat() -> np.ndarray:
    """Constant [C, N] matrix with 1/(2D) folded in: ef_m = Phat @ s_m."""
    p = np.zeros((C, N), dtype=np.float32)
    for i in range(N):
        for j in range(N):
            r = i * N + j
            p[r, i] += 1.0
            p[r, j] += 2.0
            if i == j:
                p[r, i] -= 1.0
    return p / (2.0 * D)


def _build_nc() -> bass.Bass:
    nc = bass.Bass()
    x_d = nc.declare_dram_parameter("x", [M, N, D], F32, isOutput=False)
    w1t_d = nc.declare_dram_parameter("w1t", [C, HID], BF16, isOutput=False)
    w2t_d = nc.declare_dram_parameter("w2t", [HID, C], BF16, isOutput=False)
    ph_d = nc.declare_dram_parameter("phat", [C, N], BF16, isOutput=False)
    id_d = nc.declare_dram_parameter("ident", [128, 128], F32, isOutput=False)
    out_d = nc.declare_dram_parameter("out", [M, N, D], F32, isOutput=True)
    dbg = {}
    if not DEBUG:
        pass
    else:
     for nm, shp in [("dbg_sall", [128, NGRP]), ("dbg_sv", [N, M]),
                    ("dbg_es", [M, C]), ("dbg_covd", [NGRP, 128, N]),
                    ("dbg_maskT", [NGRP, 128, N]), ("dbg_egT", [NGRP, 128, N]),
                    ("dbg_expT", [NGRP, 128, N]), ("dbg_srow", [NGRP, 1, 128]),
                    ("dbg_xTs", [128, 256])]:
        dbg[nm] = nc.declare_dram_parameter(nm, shp, F32, isOutput=True)

    s_dram = nc.dram_tensor("s_scratch", [M, N], F32)
    e_dram = nc.dram_tensor("e_scratch", [M, C], F32)

    ctx = ExitStack()
    with ctx:
        def sb(name, shape, dt=F32):
            return ctx.enter_context(nc.sbuf_tensor(name, shape, dt))

        def ps(name, shape):
            return ctx.enter_context(nc.psum_tensor(name, shape, F32))

        # ---- SBUF ----
        ident = sb("sb_ident", [128, 128])
        psb = sb("sb_ph", [128, 8 * N], BF16)           # Phat K-chunks (lhsT)
        w1 = [sb(f"w1_{i}", [128, HID], BF16) for i in range(8)]
        w2 = [sb(f"w2_{i}", [128, C], BF16) for i in range(4)]
        xt = [sb(f"xt{i}", [128, D + 1]) for i in range(NGRP)]   # x | ones
        xTs = [sb(f"xTs{i}", [128, 256]) for i in range(NGRP)]   # x transposed
        s_all = sb("s_all", [128, NGRP])
        srow = [sb(f"srow{i}", [1, 128]) for i in range(NGRP)]   # s as a row
        sneg = [sb(f"sneg{i}", [1, 128]) for i in range(NGRP)]   # -s/D row
        covd = [sb(f"covd{i}", [128, N]) for i in range(NGRP)]
        maskT = [sb(f"maskT{i}", [128, N]) for i in range(NGRP)]
        sv = sb("sv", [N, M])
        svb = sb("svb", [N, M], BF16)
        ut = sb("ut", [N, HID], BF16)                   # (W1 @ Phat).T
        hs = sb("hs", [128, 4 * M], BF16)
        es = sb("es", [M, C])
        egT = [sb(f"egT{i}", [128, N]) for i in range(NGRP)]
        expT = [sb(f"expT{i}", [128, N]) for i in range(NGRP)]
        pg = sb("pg", [128, N])                         # exp(e) * mask
        asoft = sb("asoft", [128, 128])                 # block-diag p.T
        rinv = sb("rinv", [128, 1])
        osb = [sb(f"osb{i}", [128, D]) for i in range(NGRP)]

        # ---- PSUM (7 banks) ----
        pt0 = ps("pt0", [128, 128])       # transpose ping
        pt1 = ps("pt1", [128, 128])       # transpose pong
        covp = ps("covp", [128, 128])     # cov accumulation
        utp = ps("utp", [N, HID])         # U.T accumulation
        hp = ps("hp", [128, M])           # h chunk
        e2p = ps("e2p", [M, 512])         # e2.T half
        outp = ps("outp", [128, D + 1])   # out group | row sums

        DX = nc.alloc_semaphore(name="DX")    # x loads (sync q)
        DI = nc.alloc_semaphore(name="DI")    # ident (sync q)
        DP = nc.alloc_semaphore(name="DP")    # phat (scalar q)
        D1 = nc.alloc_semaphore(name="D1")    # w1 (scalar q)
        D2 = nc.alloc_semaphore(name="D2")    # w2 (sync q)
        DG = nc.alloc_semaphore(name="DG")    # gpsimd SWDGE chain (in-order)
        DO = nc.alloc_semaphore(name="DO")    # output stores
        PEs = nc.alloc_semaphore(name="PE")
        VEs = nc.alloc_semaphore(name="VE")
        SEs = nc.alloc_semaphore(name="SE")
        sems = {"DX": DX, "DI": DI, "DP": DP, "D1": D1, "D2": D2,
                "DG": DG, "DO": DO, "PE": PEs, "VE": VEs, "SE": SEs}

        block_cm = nc.Block()
        with block_cm as block:
            # program items: (engine, sem, inc, fn, ck, waits, dr)
            # ck => sem update (for compute engines: riding on a drain).
            # dr => plain drain after the op (same-engine RAW safety), no inc.
            prog = []

            def op(eng, semn, fn, ck=None, waits=(), dr=False):
                prog.append((eng, semn, 1 if ck else 0, fn, ck, list(waits), dr))

            def dma(eng, semn, fn, ck=None, waits=()):
                prog.append((eng, semn, 16, fn, ck, list(waits), False))

            def ncdma(fn):
                with nc.allow_non_contiguous_dma(reason="tiny transpose bounce"):
                    return fn()

            # ---------- sync engine DMAs: x, ident, w2, outputs ----------
            for g in range(NGRP):
                dma("sync", "DX",
                    lambda g=g: nc.sync.dma_start(
                        out=xt[g][:, 0:D],
                        in_=x_d[g * MG:(g + 1) * MG].rearrange("m n d -> (m n) d")),
                    ck=f"DX{g}")
            dma("sync", "DI",
                lambda: nc.sync.dma_start(out=ident[:], in_=id_d[:]), ck="DWI")
            for k in range(4):
                dma("sync", "D2",
                    lambda k=k: nc.sync.dma_start(
                        out=w2[k][:], in_=w2t_d[k * 128:(k + 1) * 128, :]),
                    ck=f"DW2_{k}")

            # ---------- scalar engine DMAs: phat, w1 (second HWDGE queue) ----
            for k in range(8):
                dma("scalar", "DP",
                    lambda k=k: nc.scalar.dma_start(
                        out=psb[:, k * N:(k + 1) * N],
                        in_=ph_d[k * 128:(k + 1) * 128, :]),
                    ck=f"DPH{k}")
            for k in range(8):
                dma("scalar", "D1",
                    lambda k=k: nc.scalar.dma_start(
                        out=w1[k][:], in_=w1t_d[k * 128:(k + 1) * 128, :]),
                    ck=f"DW1_{k}")

            # ---------- vector: init memsets, row sums ----------
            for g in range(NGRP):
                op("vector", "VE",
                   lambda g=g: nc.vector.memset(xt[g][:, D:D + 1], 1.0))
            op("vector", "VE", lambda: nc.vector.memset(asoft[:], 0.0),
               ck="V_INIT")
            for g in range(NGRP):
                op("vector", "VE",
                   lambda g=g: nc.vector.reduce_sum(
                       out=s_all[:, g:g + 1], in_=xt[g][:, 0:D], axis=X),
                   ck="V_RED" if g == NGRP - 1 else None,
                   waits=[("DX", f"DX{NGRP-1}")] if g == 0 else ())

            # ---------- gpsimd: s bounce, srow loads, sv ----------
            dma("gpsimd", "DG",
                lambda: ncdma(lambda: nc.gpsimd.dma_start(
                    out=s_dram.rearrange("(g ml) n -> (ml n) g", g=NGRP),
                    in_=s_all[:])),
                ck="DG_S", waits=[("VE", "V_RED")])
            for g in range(NGRP):
                dma("gpsimd", "DG",
                    lambda g=g: nc.gpsimd.dma_start(
                        out=srow[g][0:1, :],
                        in_=s_dram[g * MG:(g + 1) * MG, :].rearrange(
                            "m n -> (m n)")),
                    ck=f"DG_SR{g}",
                    waits=[("DG", "DG_S")] if g == 0 else ())
            dma("gpsimd", "DG",
                lambda: ncdma(lambda: nc.gpsimd.dma_start(
                    out=sv[:], in_=s_dram.rearrange("m n -> n m"))),
                ck="DG_SV")

            # ---------- tensor: transposes, cov, MLP, attention ----------
            def emit_cov_group(g):
                w = [("DX", f"DX{NGRP-1}"), ("DI", "DWI")] if g == 0 else \
                    [("VE", f"V_XT{g-1}")]       # pt0/pt1 bank reuse
                for k, pt in ((0, pt0), (1, pt1)):
                    op("tensor", "PE",
                       lambda g=g, k=k, pt=pt: nc.tensor.transpose(
                           out=pt[:], in_=xt[g][:, k * 128:(k + 1) * 128],
                           identity=ident[:]),
                       ck=f"P_TP{g}_{k}", waits=w if k == 0 else ())
                w = [("VE", f"V_XT{g}")]
                if g > 0:
                    w.append(("VE", f"V_COVD{g-1}"))  # covp bank reuse
                for k in range(2):
                    op("tensor", "PE",
                       lambda g=g, k=k: nc.tensor.matmul(
                           out=covp[:],
                           lhsT=xTs[g][:, k * 128:(k + 1) * 128],
                           rhs=xTs[g][:, k * 128:(k + 1) * 128],
                           start=(k == 0), stop=False),
                       waits=w if k == 0 else ())
                op("tensor", "PE",
                   lambda g=g: nc.tensor.matmul(
                       out=covp[:], lhsT=sneg[g][0:1, :], rhs=srow[g][0:1, :],
                       start=False, stop=True),
                   ck=f"P_COV{g}", waits=[("VE", f"V_SNEG{g}")])

            emit_cov_group(0)
            # UT = (W1 @ Phat).T, bf16
            for k in range(8):
                op("tensor", "PE",
                   lambda k=k: nc.tensor.matmul(
                       out=utp[:], lhsT=psb[:, k * N:(k + 1) * N], rhs=w1[k][:],
                       start=(k == 0), stop=(k == 7)),
                   ck="P_UT" if k == 7 else None,
                   waits=[("DP", "DPH7"), ("D1", "DW1_7")] if k == 0 else ())
            # h = relu(U @ s.T) in 4 chunks
            for oc in range(4):
                w = [("VE", "V_UT"), ("VE", "V_SV")] if oc == 0 else \
                    [("SE", f"S_RELU{oc-1}")]         # hp bank reuse
                op("tensor", "PE",
                   lambda oc=oc: nc.tensor.matmul(
                       out=hp[:], lhsT=ut[:, oc * 128:(oc + 1) * 128],
                       rhs=svb[:], start=True, stop=True),
                   ck=f"P_H{oc}", waits=w)
                op("scalar", "SE",
                   lambda oc=oc: nc.scalar.activation(
                       out=hs[:, oc * M:(oc + 1) * M], in_=hp[:], func=AF.Relu),
                   ck=f"S_RELU{oc}", waits=[("PE", f"P_H{oc}")])
            emit_cov_group(1)
            # e2.T halves + sigmoid
            for hf in range(2):
                for k in range(4):
                    w = []
                    if hf == 0 and k == 0:
                        w = [("SE", "S_RELU3"), ("D2", "DW2_3")]
                    elif k == 0:
                        w = [("SE", "S_SIG0")]        # e2p bank reuse
                    op("tensor", "PE",
                       lambda hf=hf, k=k: nc.tensor.matmul(
                           out=e2p[:], lhsT=hs[:, k * M:(k + 1) * M],
                           rhs=w2[k][:, hf * 512:(hf + 1) * 512],
                           start=(k == 0), stop=(k == 3)),
                       ck=f"P_E2{hf}" if k == 3 else None, waits=w)
                op("scalar", "SE",
                   lambda hf=hf: nc.scalar.activation(
                       out=es[:, hf * 512:(hf + 1) * 512], in_=e2p[:],
                       func=AF.Sigmoid),
                   ck=f"S_SIG{hf}", waits=[("PE", f"P_E2{hf}")])
            emit_cov_group(2)
            # attention: p.T block-diag matmul with ones-column row sums
            for g in range(NGRP):
                w = [("VE", f"V_ASOFT{g}")]
                if g == 0:
                    w.append(("VE", "V_INIT"))
                else:
                    w.append(("VE", f"V_OSB{g-1}"))   # outp bank reuse
                op("tensor", "PE",
                   lambda g=g: nc.tensor.matmul(
                       out=outp[:], lhsT=asoft[:], rhs=xt[g][:, 0:D + 1],
                       start=True, stop=True),
                   ck=f"P_OUT{g}", waits=w)

            # ---------- gpsimd: e bounce ----------
            dma("gpsimd", "DG",
                lambda: nc.gpsimd.dma_start(out=e_dram[:], in_=es[:]),
                ck="DG_E", waits=[("SE", "S_SIG1")])
            for g in range(NGRP):
                dma("gpsimd", "DG",
                    lambda g=g: nc.gpsimd.dma_start(
                        out=egT[g][:],
                        in_=e_dram[g * MG:(g + 1) * MG, :].rearrange(
                            "m (i j) -> (m i) j", i=N)),
                    ck=f"DG_EG{g}",
                    waits=[("DG", "DG_E")] if g == 0 else ())

            # ---------- scalar: exp ----------
            for g in range(NGRP):
                op("scalar", "SE",
                   lambda g=g: nc.scalar.activation(
                       out=expT[g][:], in_=egT[g][:], func=AF.Exp),
                   ck=f"S_EXP{g}", waits=[("DG", f"DG_EG{NGRP-1}")])

            # ---------- vector: everything else, in execution order ----------
            for g in range(NGRP):
                for k in range(2):
                    op("vector", "VE",
                       lambda g=g, k=k: nc.vector.tensor_copy(
                           out=xTs[g][:, k * 128:(k + 1) * 128],
                           in_=(pt0 if k == 0 else pt1)[:]),
                       ck=f"V_XT{g}" if k == 1 else None,
                       waits=[("PE", f"P_TP{g}_{k}")])
                if g == 0:
                    for gg in range(NGRP):
                        op("vector", "VE",
                           lambda gg=gg: nc.vector.tensor_scalar_mul(
                               out=sneg[gg][0:1, :], in0=srow[gg][0:1, :],
                               scalar1=-1.0 / D),
                           ck=f"V_SNEG{gg}", waits=[("DG", "DG_SV")])
                    op("vector", "VE",
                       lambda: nc.vector.tensor_copy(out=svb[:], in_=sv[:]),
                       ck="V_SV", waits=[("DG", "DG_SV")])
                for ml in range(MG):
                    r = slice(ml * N, (ml + 1) * N)
                    op("vector", "VE",
                       lambda g=g, r=r: nc.vector.tensor_copy(
                           out=covd[g][r, :], in_=covp[r, r]),
                       ck=f"V_COVD{g}" if ml == MG - 1 else None,
                       waits=[("PE", f"P_COV{g}")] if ml == 0 else ())
                op("vector", "VE",
                   lambda g=g: nc.vector.tensor_scalar(
                       out=maskT[g][:], in0=covd[g][:], scalar1=0.0,
                       scalar2=None, op0=mybir.AluOpType.is_gt),
                   ck=f"V_MASK{g}")
                if g == 0:
                    op("vector", "VE",
                       lambda: nc.vector.tensor_copy(out=ut[:], in_=utp[:]),
                       ck="V_UT", waits=[("PE", "P_UT")])
            for g in range(NGRP):
                w = [("SE", f"S_EXP{g}")]
                if g > 0:
                    w.append(("PE", f"P_OUT{g-1}"))   # asoft WAR vs prev matmul
                op("vector", "VE",
                   lambda g=g: nc.vector.tensor_tensor(
                       out=pg[:], in0=expT[g][:], in1=maskT[g][:],
                       op=mybir.AluOpType.mult),
                   waits=w, dr=True)
                for ml in range(MG):
                    r = slice(ml * N, (ml + 1) * N)
                    op("vector", "VE",
                       lambda g=g, r=r, ml=ml: nc.vector.transpose(
                           out=asoft[r, ml * N:(ml + 1) * N], in_=pg[r, :]),
                       ck=f"V_ASOFT{g}" if ml == MG - 1 else None)
                op("vector", "VE",
                   lambda g=g: nc.vector.reciprocal(
                       out=rinv[:], in_=outp[:, D:D + 1]),
                   waits=[("PE", f"P_OUT{g}")], dr=True)
                op("vector", "VE",
                   lambda g=g: nc.vector.tensor_scalar_mul(
                       out=osb[g][:], in0=outp[:, 0:D], scalar1=rinv[:]),
                   ck=f"V_OSB{g}")
                dma("sync", "DO",
                    lambda g=g: nc.sync.dma_start(
                        out=out_d[g * MG:(g + 1) * MG].rearrange(
                            "m n d -> (m n) d"),
                        in_=osb[g][:]),
                    ck=f"DO{g}", waits=[("VE", f"V_OSB{g}")])
            ndbg = 0
            first = True
            dsrc0 = [] if not DEBUG else [("dbg_sall", s_all), ("dbg_sv", sv), ("dbg_es", es),
                          ("dbg_xTs", xTs[0])]
            for nm, t in dsrc0:
                w = [("VE", f"V_OSB{NGRP-1}"), ("SE", f"S_EXP{NGRP-1}")] if first else ()
                first = False
                dma("sync", "DO",
                    lambda nm=nm, t=t: nc.sync.dma_start(
                        out=dbg[nm][:], in_=t[:]), waits=w)
                ndbg += 1
            for g in range(NGRP if DEBUG else 0):
                for nm, lst in [("dbg_covd", covd), ("dbg_maskT", maskT),
                                ("dbg_egT", egT), ("dbg_expT", expT),
                                ("dbg_srow", srow)]:
                    dma("sync", "DO",
                        lambda nm=nm, lst=lst, g=g: nc.sync.dma_start(
                            out=dbg[nm][g], in_=lst[g][:]))
                    ndbg += 1
            prog.append(("sync", "DO", 0, None, None,
                         [("DO", 16 * (NGRP + ndbg))], False))

            # ---------- pass 1: resolve checkpoint values ----------
            counts = {k: 0 for k in sems}
            cks = {}
            for eng, semn, inc, fn, ck, waits, dr in prog:
                counts[semn] += inc
                if ck:
                    cks[ck] = counts[semn]

            def resolve(v):
                return cks[v] if isinstance(v, str) else v

            # ---------- pass 2: emit per-engine ----------
            def emit_for(eng_name, eng):
                for peng, semn, inc, fn, ck, waits, dr in prog:
                    if peng != eng_name:
                        continue
                    for wsem, wval in waits:
                        eng.wait_ge(sems[wsem], resolve(wval))
                    if fn is None:
                        continue
                    inst = fn()
                    if inc and semn in ("PE", "VE", "SE"):
                        eng.drain().then_inc(sems[semn], inc)
                    elif inc:
                        inst.then_inc(sems[semn], inc)
                    elif dr:
                        eng.drain()

            @block.sync
            def _(eng):
                emit_for("sync", eng)

            @block.vector
            def _(eng):
                emit_for("vector", eng)

            @block.scalar
            def _(eng):
                emit_for("scalar", eng)

            @block.tensor
            def _(eng):
                emit_for("tensor", eng)

            @block.gpsimd
            def _(eng):
                emit_for("gpsimd", eng)

    return nc


_NC = None


def _get_nc() -> bass.Bass:
    global _NC
    if _NC is None:
        _NC = _build_nc()
    return _NC


def _run(x, W1, W2, trace=False, n_cores=B):
    nc = _get_nc()
    x = np.ascontiguousarray(np.asarray(x, dtype=np.float32))
    w1t = np.ascontiguousarray(
        np.asarray(W1, dtype=np.float32).T).astype(ml_dtypes.bfloat16)
    w2t = np.ascontiguousarray(
        np.asarray(W2, dtype=np.float32).T).astype(ml_dtypes.bfloat16)
    phat = _phat().astype(ml_dtypes.bfloat16)
    ident = np.eye(128, dtype=np.float32)
    in_maps = [
        {"x": x[b], "w1t": w1t, "w2t": w2t, "phat": phat, "ident": ident}
        for b in range(n_cores)
    ]
    res = run_bass_kernel_spmd(nc, in_maps, list(range(n_cores)), trace=trace)
    out = np.stack([res.results[b]["out"] for b in range(n_cores)])
    return out, res


def kernel(x, W1, W2):
    out, _ = _run(x, W1, W2)
    return out
